# revision 13
# baseline (speedup 1.0000x reference)
"""Trainium2 Bass kernel for the LELoss problem (raw Bass, 8-core SPMD).

loss = mean_b ||x_b - dec_b||^2
     + 1.1 * mean_b ||enc_b - (lat @ rsrA.T)_b||^2
     + 0.1 * mean((rsrA.T @ rsrA - I)^2)

(The knn/cdist/topk in the original module is dead code - its result is never
used - so the returned loss reduces to the three terms above.)

v8: the element-wise subtraction happens in the DMA path.  x streams as
fp8e4 over the two HWDGE queues into SBUF; dec streams SIGN-FLIPPED (a
lossless fp8 sign-bit repack on the host) over the gpsimd SWDGE queue with
accum_op=add, so the DMA engines deposit d = x - dec directly.  No vector
subs remain: the DVE only does the tiny PCA/proj reductions and three of
the eight squares; the ACT engine squares the rest (activation Square +
accumulator).  sum(enc^2) and sum(rsrA^2) come free from the PE as
<psum, I> traces (psum_E = enc^T enc accumulated alongside the M matmuls,
and trace(G)).  Measured rel err ~1e-3 vs the 2e-2 gate (fp8 inputs + fp8
difference).

Streams: x8 2 x 512KB per HWDGE queue (2KB lines), neg-dec 8 x 128KB on
the SWDGE queue (each gated on its x chunk), pack (enc/lat/rsrA/identity,
bf16) first on the SWDGE queue.  Total 2.4MB/core vs 8.6MB for f32.

Partial sums land in columns of a [128,16] SBUF accumulator S whose first
13 columns are DMA'd out per core; the host collapses partitions/cores and
applies the weights:
  sum||enc - lat@A.T||^2 = sum(enc^2) - 2*sum(M .* A) + sum(L .* G)
  sum((G - I)^2) = sum(G^2) - 2*sum(A^2) + I_dim
"""

import contextlib

import ml_dtypes
import numpy as np

try:
    import concourse.bass as bass
except ImportError:  # pragma: no cover - grading env fallback
    import sys

    sys.path.insert(0, "/opt/trn_rl_repo")
    import concourse.bass as bass

from concourse import mybir
from concourse.bass_utils import run_bass_kernel_spmd

N_CORES = 8
B, D, E, I = 8192, 1024, 128, 20
R = B // N_CORES  # rows per core = 1024
P = 128  # SBUF partitions
RT = R // P  # row tiles per core = 8
W = RT * D  # packed stream width = 8192
S_COLS = 16
OUT_COLS = 13
F32 = mybir.dt.float32
BF16 = mybir.dt.bfloat16
FP8 = mybir.dt.float8e4
BF = ml_dtypes.bfloat16
F8 = ml_dtypes.float8_e4m3

ENC_W = RT * E  # 1024 cols of packed enc
LAT_W = RT * I  # 160 cols of packed lat
ID_OFF = ENC_W + LAT_W + I  # 1204
PACK_W = ID_OFF + P  # 1332 (identity appended)

XCHUNKS = [(0, 2048), (2048, 4096), (4096, 6144), (6144, 8192)]
NCHUNKS = [(k * 1024, (k + 1) * 1024) for k in range(8)]
N_ACT_SQ = 5  # nd chunks 0..4 squared on ACT, 5..7 on DVE

TRACE = False
LAST_RESULT = None

_NC = None


def _build_nc():
    nc = bass.Bass()
    x8 = nc.dram_tensor("x8", [P, W], FP8, kind="ExternalInput")
    nd8 = nc.dram_tensor("nd8", [P, W], FP8, kind="ExternalInput")
    pack = nc.dram_tensor("pack", [P, PACK_W], BF16, kind="ExternalInput")
    out = nc.dram_tensor("out", [P, OUT_COLS], F32, kind="ExternalOutput")

    mult = mybir.AluOpType.mult
    add = mybir.AluOpType.add
    bypass = mybir.AluOpType.bypass
    Square = mybir.ActivationFunctionType.Square

    ctx = contextlib.ExitStack()
    with ctx:
        dd = ctx.enter_context(nc.sbuf_tensor("dd", [P, W], FP8))
        pk = ctx.enter_context(nc.sbuf_tensor("pk", [P, PACK_W], BF16))
        S = ctx.enter_context(nc.sbuf_tensor("S", [P, S_COLS], F32))
        G_sb = ctx.enter_context(nc.sbuf_tensor("G_sb", [I, I], F32))
        scr_m = ctx.enter_context(nc.sbuf_tensor("scr_m", [E, I], F32))
        scr_g = ctx.enter_context(nc.sbuf_tensor("scr_g", [I, I], F32))
        scr_e = ctx.enter_context(nc.sbuf_tensor("scr_e", [P, P], F32))

        psum_M = ctx.enter_context(nc.psum_tensor([E, I], F32))
        psum_L = ctx.enter_context(nc.psum_tensor([I, I], F32))
        psum_G = ctx.enter_context(nc.psum_tensor([I, I], F32))
        psum_E = ctx.enter_context(nc.psum_tensor([P, P], F32))

        s_x = [ctx.enter_context(nc.semaphore(f"s_x{k}")) for k in range(4)]
        s_n = [ctx.enter_context(nc.semaphore(f"s_n{k}")) for k in range(8)]
        s_pk = ctx.enter_context(nc.semaphore("s_pk"))
        s_init = ctx.enter_context(nc.semaphore("s_init"))
        s_pe = ctx.enter_context(nc.semaphore("s_pe"))
        s_sqA = ctx.enter_context(nc.semaphore("s_sqA"))
        s_vfin = ctx.enter_context(nc.semaphore("s_vfin"))
        s_out = ctx.enter_context(nc.semaphore("s_out"))

        block = ctx.enter_context(nc.Block())

        def enc_t(t):
            return pk[:, t * E : (t + 1) * E]

        def lat_t(t):
            return pk[:, ENC_W + t * I : ENC_W + (t + 1) * I]

        rsra = pk[:, ENC_W + LAT_W : ID_OFF]
        ident = pk[:, ID_OFF:PACK_W]

        @block.sync
        def _(sync):
            for k in (0, 2):
                lo, hi = XCHUNKS[k]
                sync.dma_start(out=dd[:, lo:hi], in_=x8[:, lo:hi]).then_inc(
                    s_x[k], 16
                )
            sync.wait_ge(s_sqA, N_ACT_SQ)
            sync.wait_ge(s_vfin, 1)
            sync.dma_start(out=out[:, :], in_=S[:, 0:OUT_COLS]).then_inc(s_out, 16)
            sync.wait_ge(s_out, 16)

        @block.scalar
        def _(scalar):
            for k in (1, 3):
                lo, hi = XCHUNKS[k]
                scalar.dma_start(out=dd[:, lo:hi], in_=x8[:, lo:hi]).then_inc(
                    s_x[k], 16
                )
            # dummy activation: hoists the ACT_TABLE_LOAD into idle time
            nc.scalar.activation(
                out=scr_g[:1, 0:1], in_=scr_g[:1, 0:1], func=Square
            )
            scalar.wait_ge(s_init, 1)
            for k in range(N_ACT_SQ):
                lo, hi = NCHUNKS[k]
                scalar.wait_ge(s_n[k], 16)
                nc.scalar.activation(
                    out=dd[:, lo:hi], in_=dd[:, lo:hi], func=Square,
                    accum_out=S[:, k : k + 1],
                ).then_inc(s_sqA, 1)

        @block.vector
        def _(vector):
            nc.vector.memset(S[:, :], 0.0).then_inc(s_init, 1)
            # PCA/proj reductions + identity traces for enc^2 / rsrA^2
            vector.wait_ge(s_pe, 1)
            nc.vector.tensor_copy(G_sb[:, :], psum_G[:, :])
            nc.vector.scalar_tensor_tensor(
                out=scr_m[:, :], in0=psum_M[:, :], scalar=1.0, in1=rsra,
                op0=bypass, op1=mult, accum_out=S[:E, 9:10],
            )
            nc.vector.scalar_tensor_tensor(
                out=scr_g[:, :], in0=psum_L[:, :], scalar=1.0, in1=G_sb[:, :],
                op0=bypass, op1=mult, accum_out=S[:I, 10:11],
            )
            nc.vector.scalar_tensor_tensor(
                out=scr_g[:, :], in0=G_sb[:, :], scalar=1.0, in1=G_sb[:, :],
                op0=bypass, op1=mult, accum_out=S[:I, 11:12],
            )
            nc.vector.scalar_tensor_tensor(
                out=scr_e[:, :], in0=psum_E[:, :], scalar=1.0, in1=ident,
                op0=bypass, op1=mult, accum_out=S[:, 8:9],
            )
            nc.vector.scalar_tensor_tensor(
                out=scr_g[:, :], in0=psum_G[:, :], scalar=1.0, in1=ident[:I, 0:I],
                op0=bypass, op1=mult, accum_out=S[:I, 12:13],
            )
            # squares of the last nd chunks (stt, in place over fp8 d)
            last = None
            for k in range(N_ACT_SQ, 8):
                lo, hi = NCHUNKS[k]
                vector.wait_ge(s_n[k], 16)
                last = nc.vector.scalar_tensor_tensor(
                    out=dd[:, lo:hi], in0=dd[:, lo:hi], scalar=1.0,
                    in1=dd[:, lo:hi], op0=bypass, op1=mult,
                    accum_out=S[:, k : k + 1],
                )
            last.then_inc(s_vfin, 1)

        @block.gpsimd
        def _(gpsimd):
            # SWDGE queue: pack first, then the neg-dec accumulate chain
            gpsimd.dma_start(out=pk[:, :], in_=pack[:, :]).then_inc(s_pk, 16)
            for k, (lo, hi) in enumerate(NCHUNKS):
                gpsimd.wait_ge(s_x[k // 2], 16)
                gpsimd.dma_start(
                    out=dd[:, lo:hi], in_=nd8[:, lo:hi], accum_op=add
                ).then_inc(s_n[k], 16)

        @block.tensor
        def _(tensor):
            tensor.wait_ge(s_pk, 16)
            for t in range(RT):
                nc.tensor.matmul(
                    psum_M[:, :], lhsT=enc_t(t), rhs=lat_t(t),
                    start=(t == 0), stop=(t == RT - 1),
                )
                nc.tensor.matmul(
                    psum_E[:, :], lhsT=enc_t(t), rhs=enc_t(t),
                    start=(t == 0), stop=(t == RT - 1),
                )
            for t in range(RT):
                nc.tensor.matmul(
                    psum_L[:, :], lhsT=lat_t(t), rhs=lat_t(t),
                    start=(t == 0), stop=(t == RT - 1),
                )
            nc.tensor.matmul(
                psum_G[:, :], lhsT=rsra, rhs=rsra, start=True, stop=True
            ).then_inc(s_pe, 1)

    return nc


def kernel(x, encoded, latent, decoded, rsrA):
    global _NC, LAST_RESULT
    if _NC is None:
        _NC = _build_nc()

    x = np.ascontiguousarray(x, dtype=np.float32)
    decoded = np.ascontiguousarray(decoded, dtype=np.float32)
    encoded = np.ascontiguousarray(encoded, dtype=np.float32).astype(BF)
    latent = np.ascontiguousarray(latent, dtype=np.float32).astype(BF)
    rsrA_b = np.ascontiguousarray(rsrA, dtype=np.float32).astype(BF)
    ident = np.eye(P, dtype=np.float32).astype(BF)

    def stream_pack(a):
        # [1024, 1024] -> [128, 8192]: partition p holds row p of each of the
        # 8 row-tiles, concatenated (pure layout, any permutation works for
        # the elementwise sum-of-squares)
        return np.ascontiguousarray(
            a.reshape(RT, P, D).transpose(1, 0, 2).reshape(P, W)
        )

    in_maps = []
    for c in range(N_CORES):
        sl = slice(c * R, (c + 1) * R)
        pk = np.concatenate(
            [
                encoded[sl].reshape(P, ENC_W),
                latent[sl].reshape(P, LAT_W),
                rsrA_b,
                ident,
            ],
            axis=1,
        )
        xs = stream_pack(x[sl]).astype(F8)
        # dec ships sign-flipped (lossless fp8 sign-bit repack) so the
        # SWDGE accumulate-add computes x - dec in the DMA engines
        nds = stream_pack(decoded[sl]).astype(F8)
        nds = (nds.view(np.uint8) ^ 0x80).view(F8)
        in_maps.append(
            {
                "x8": np.ascontiguousarray(xs),
                "nd8": np.ascontiguousarray(nds),
                "pack": np.ascontiguousarray(pk),
            }
        )

    res = run_bass_kernel_spmd(_NC, in_maps, core_ids=list(range(N_CORES)), trace=TRACE)
    LAST_RESULT = res

    o = np.stack([r["out"] for r in res.results]).astype(np.float64)  # [8,128,13]
    recon = o[:, :, 0:8].sum()
    enc2 = o[:, :, 8].sum()
    cross = o[:, :, 9].sum()
    zsq = o[:, :, 10].sum()
    g2 = o[0, :, 11].sum()
    ra2 = o[0, :, 12].sum()

    pca_sq = enc2 - 2.0 * cross + zsq
    proj_sq = g2 - 2.0 * ra2 + float(I)
    loss = recon / B + 1.1 * pca_sq / B + 0.1 * proj_sq / (I * I)
    return np.asarray(loss, dtype=np.float32)


# revision 14
# speedup vs baseline: 1.2506x; 1.2506x over previous
"""Trainium2 Bass kernel for the LELoss problem (raw Bass, 8-core SPMD).

loss = mean_b ||x_b - dec_b||^2
     + 1.1 * mean_b ||enc_b - (lat @ rsrA.T)_b||^2
     + 0.1 * mean((rsrA.T @ rsrA - I)^2)

(The knn/cdist/topk in the original module is dead code - its result is never
used - so the returned loss reduces to the three terms above.)

v6 design, driven by measured rates (DVE 1x = 1.04ns/col, DVE tensor_tensor
bf16 = 2x, ACT activation ~0.98ns/col, two-source stt always 1x, per-queue
DMA throughput collapses for sub-2KB partition lines):

- The x/dec stream is split by dtype: the first 5120 columns as fp8e4
  (quarter traffic, subs at 1x, scheduled early under the stream), the last
  3072 as bf16 (2x subs, short tail).  Overall rel err ~2.6e-4 vs 2e-2.
- Each chunk ships as ONE DMA of a host-packed [x_chunk | dec_chunk] block,
  alternating between the two HWDGE queues (SP/ACT): half the DMA issues,
  single-semaphore chunks, and every partition line >= 2KB.
- enc/lat/rsrA/identity ride in one pack DMA on the gpsimd SWDGE queue so
  neither HWDGE queue nor engine pays for it.
- sum(enc^2) and sum(rsrA^2) are free on the PE: psum_E = sum_t enc_t^T
  enc_t (reusing the M-matmul operands) and trace(G); the DVE extracts
  both as <psum, Identity> products with a shipped bf16 identity.
- Squares: ACT takes the fp8 chunks + the first bf16 chunk (activation
  Square+accum chasing the DVE subs), DVE takes the last bf16 cols in one
  stt so the post-stream tail has no cross-engine hop.

Partial sums land in columns of a [128,16] SBUF accumulator S whose first
12 columns are DMA'd out per core; the host collapses partitions/cores and
applies the weights:
  sum||enc - lat@A.T||^2 = sum(enc^2) - 2*sum(M .* A) + sum(L .* G)
  sum((G - I)^2) = sum(G^2) - 2*sum(A^2) + I_dim
"""

import contextlib

import ml_dtypes
import numpy as np

try:
    import concourse.bass as bass
except ImportError:  # pragma: no cover - grading env fallback
    import sys

    sys.path.insert(0, "/opt/trn_rl_repo")
    import concourse.bass as bass

from concourse import mybir
from concourse.bass_utils import run_bass_kernel_spmd

N_CORES = 8
B, D, E, I = 8192, 1024, 128, 20
R = B // N_CORES  # rows per core = 1024
P = 128  # SBUF partitions
RT = R // P  # row tiles per core = 8
W = RT * D  # packed stream width = 8192
FCOLS = 5120  # fp8 column count
ACOLS = W - FCOLS  # bf16 column count = 3072
S_COLS = 16
OUT_COLS = 12
F32 = mybir.dt.float32
BF16 = mybir.dt.bfloat16
FP8 = mybir.dt.float8e4
BF = ml_dtypes.bfloat16
F8 = ml_dtypes.float8_e4m3

ENC_W = RT * E  # 1024 cols of packed enc
LAT_W = RT * I  # 160 cols of packed lat
ID_OFF = ENC_W + LAT_W + I  # 1204
PACK_W = ID_OFF + P  # 1332 (identity appended)

# chunk column ranges per dtype region; [x|dec] blocks live at [2lo:2hi]
FCHUNKS = [(0, 1024), (1024, 3072), (3072, 5120)]
BCHUNKS = [(0, 1024), (1024, 2048), (2048, 3072)]

TRACE = False
LAST_RESULT = None

_NC = None


def _build_nc():
    nc = bass.Bass()
    s8 = nc.dram_tensor("s8", [P, 2 * FCOLS], FP8, kind="ExternalInput")
    s16 = nc.dram_tensor("s16", [P, 2 * ACOLS], BF16, kind="ExternalInput")
    pack = nc.dram_tensor("pack", [P, PACK_W], BF16, kind="ExternalInput")
    out = nc.dram_tensor("out", [P, OUT_COLS], F32, kind="ExternalOutput")

    mult = mybir.AluOpType.mult
    sub = mybir.AluOpType.subtract
    bypass = mybir.AluOpType.bypass
    Square = mybir.ActivationFunctionType.Square

    ctx = contextlib.ExitStack()
    with ctx:
        s8q = ctx.enter_context(nc.sbuf_tensor("s8q", [P, 2 * FCOLS], FP8))
        dd8 = ctx.enter_context(nc.sbuf_tensor("dd8", [P, FCOLS], BF16))
        s16q = ctx.enter_context(nc.sbuf_tensor("s16q", [P, 2 * ACOLS], BF16))
        pk = ctx.enter_context(nc.sbuf_tensor("pk", [P, PACK_W], BF16))
        S = ctx.enter_context(nc.sbuf_tensor("S", [P, S_COLS], F32))
        G_sb = ctx.enter_context(nc.sbuf_tensor("G_sb", [I, I], F32))
        scr_m = ctx.enter_context(nc.sbuf_tensor("scr_m", [E, I], F32))
        scr_g = ctx.enter_context(nc.sbuf_tensor("scr_g", [I, I], F32))
        scr_e = ctx.enter_context(nc.sbuf_tensor("scr_e", [P, P], F32))

        psum_M = ctx.enter_context(nc.psum_tensor([E, I], F32))
        psum_L = ctx.enter_context(nc.psum_tensor([I, I], F32))
        psum_G = ctx.enter_context(nc.psum_tensor([I, I], F32))
        psum_E = ctx.enter_context(nc.psum_tensor([P, P], F32))

        s_f = [ctx.enter_context(nc.semaphore(f"s_f{k}")) for k in range(3)]
        s_b = [ctx.enter_context(nc.semaphore(f"s_b{k}")) for k in range(3)]
        s_pk = ctx.enter_context(nc.semaphore("s_pk"))
        s_init = ctx.enter_context(nc.semaphore("s_init"))
        s_pe = ctx.enter_context(nc.semaphore("s_pe"))
        s_sub = ctx.enter_context(nc.semaphore("s_sub"))
        s_sqA = ctx.enter_context(nc.semaphore("s_sqA"))
        s_vfin = ctx.enter_context(nc.semaphore("s_vfin"))
        s_out = ctx.enter_context(nc.semaphore("s_out"))

        block = ctx.enter_context(nc.Block())

        def enc_t(t):
            return pk[:, t * E : (t + 1) * E]

        def lat_t(t):
            return pk[:, ENC_W + t * I : ENC_W + (t + 1) * I]

        rsra = pk[:, ENC_W + LAT_W : ID_OFF]
        ident = pk[:, ID_OFF:PACK_W]

        # x / dec sub-views of a combined [x|dec] chunk block
        def xpart(t, lo, hi):
            return t[:, 2 * lo : lo + hi]

        def dpart(t, lo, hi):
            return t[:, lo + hi : 2 * hi]

        @block.sync
        def _(sync):
            # queue A (slower in practice): f0, b1, b2
            lo, hi = FCHUNKS[0]
            sync.dma_start(
                out=s8q[:, 2 * lo : 2 * hi], in_=s8[:, 2 * lo : 2 * hi]
            ).then_inc(s_f[0], 16)
            for k in (1, 2):
                lo, hi = BCHUNKS[k]
                sync.dma_start(
                    out=s16q[:, 2 * lo : 2 * hi], in_=s16[:, 2 * lo : 2 * hi]
                ).then_inc(s_b[k], 16)
            sync.wait_ge(s_sqA, 5)
            sync.wait_ge(s_vfin, 1)
            sync.dma_start(out=out[:, :], in_=S[:, 0:OUT_COLS]).then_inc(s_out, 16)
            sync.wait_ge(s_out, 16)

        @block.scalar
        def _(scalar):
            # queue B (faster in practice): f1, f2, b0
            for k in (1, 2):
                lo, hi = FCHUNKS[k]
                scalar.dma_start(
                    out=s8q[:, 2 * lo : 2 * hi], in_=s8[:, 2 * lo : 2 * hi]
                ).then_inc(s_f[k], 16)
            lo, hi = BCHUNKS[0]
            scalar.dma_start(
                out=s16q[:, 2 * lo : 2 * hi], in_=s16[:, 2 * lo : 2 * hi]
            ).then_inc(s_b[0], 16)
            # dummy activation: pulls the ACT_TABLE_LOAD into this idle
            # window instead of the first square's critical path
            nc.scalar.activation(
                out=scr_g[:1, 0:1], in_=scr_g[:1, 0:1], func=Square
            )
            # squares chase the DVE subs (consumption order f0,f1,f2,b1,b0a)
            scalar.wait_ge(s_init, 1)
            for k, (lo, hi) in enumerate(FCHUNKS):
                scalar.wait_ge(s_sub, k + 1)
                nc.scalar.activation(
                    out=dd8[:, lo:hi], in_=dd8[:, lo:hi], func=Square,
                    accum_out=S[:, k : k + 1],
                ).then_inc(s_sqA, 1)
            scalar.wait_ge(s_sub, 4)
            lo, hi = BCHUNKS[1]
            nc.scalar.activation(
                out=xpart(s16q, lo, hi), in_=xpart(s16q, lo, hi), func=Square,
                accum_out=S[:, 3:4],
            ).then_inc(s_sqA, 1)
            scalar.wait_ge(s_sub, 5)
            lo, hi = BCHUNKS[0]
            half = (lo + hi) // 2
            nc.scalar.activation(
                out=xpart(s16q, lo, hi)[:, 0 : half - lo],
                in_=xpart(s16q, lo, hi)[:, 0 : half - lo], func=Square,
                accum_out=S[:, 4:5],
            ).then_inc(s_sqA, 1)

        @block.vector
        def _(vector):
            nc.vector.memset(S[:, :], 0.0).then_inc(s_init, 1)
            # fp8 subs (1x) early, overlapping the stream
            for k, (lo, hi) in enumerate(FCHUNKS):
                vector.wait_ge(s_f[k], 16)
                nc.vector.tensor_tensor(
                    dd8[:, lo:hi], xpart(s8q, lo, hi), dpart(s8q, lo, hi), op=sub
                ).then_inc(s_sub, 1)
            # PCA/proj reductions + identity traces for enc^2 / rsrA^2
            vector.wait_ge(s_pe, 1)
            nc.vector.tensor_copy(G_sb[:, :], psum_G[:, :])
            nc.vector.scalar_tensor_tensor(
                out=scr_m[:, :], in0=psum_M[:, :], scalar=1.0, in1=rsra,
                op0=bypass, op1=mult, accum_out=S[:E, 8:9],
            )
            nc.vector.scalar_tensor_tensor(
                out=scr_g[:, :], in0=psum_L[:, :], scalar=1.0, in1=G_sb[:, :],
                op0=bypass, op1=mult, accum_out=S[:I, 9:10],
            )
            nc.vector.scalar_tensor_tensor(
                out=scr_g[:, :], in0=G_sb[:, :], scalar=1.0, in1=G_sb[:, :],
                op0=bypass, op1=mult, accum_out=S[:I, 10:11],
            )
            nc.vector.scalar_tensor_tensor(
                out=scr_e[:, :], in0=psum_E[:, :], scalar=1.0, in1=ident,
                op0=bypass, op1=mult, accum_out=S[:, 7:8],
            )
            nc.vector.scalar_tensor_tensor(
                out=scr_g[:, :], in0=psum_G[:, :], scalar=1.0, in1=ident[:I, 0:I],
                op0=bypass, op1=mult, accum_out=S[:I, 11:12],
            )
            # bf16 subs (2x), in place over the x half of each block,
            # consumed in expected arrival order: b1, b0, b2
            for k in (1, 0, 2):
                lo, hi = BCHUNKS[k]
                vector.wait_ge(s_b[k], 16)
                nc.vector.tensor_tensor(
                    xpart(s16q, lo, hi), xpart(s16q, lo, hi),
                    dpart(s16q, lo, hi), op=sub,
                ).then_inc(s_sub, 1)
            # DVE squares: second half of b0 and all of b2 (ACT has the rest)
            lo, hi = BCHUNKS[0]
            half = (lo + hi) // 2
            bx = xpart(s16q, lo, hi)
            nc.vector.scalar_tensor_tensor(
                out=bx[:, half - lo : hi - lo], in0=bx[:, half - lo : hi - lo],
                scalar=1.0, in1=bx[:, half - lo : hi - lo], op0=bypass, op1=mult,
                accum_out=S[:, 6:7],
            )
            lo, hi = BCHUNKS[2]
            nc.vector.scalar_tensor_tensor(
                out=xpart(s16q, lo, hi), in0=xpart(s16q, lo, hi), scalar=1.0,
                in1=xpart(s16q, lo, hi), op0=bypass, op1=mult,
                accum_out=S[:, 5:6],
            ).then_inc(s_vfin, 1)

        @block.gpsimd
        def _(gpsimd):
            # pack rides the SWDGE queue so the HWDGE queues stay clean
            gpsimd.dma_start(out=pk[:, :], in_=pack[:, :]).then_inc(s_pk, 16)

        @block.tensor
        def _(tensor):
            tensor.wait_ge(s_pk, 16)
            for t in range(RT):
                nc.tensor.matmul(
                    psum_M[:, :], lhsT=enc_t(t), rhs=lat_t(t),
                    start=(t == 0), stop=(t == RT - 1),
                )
                nc.tensor.matmul(
                    psum_E[:, :], lhsT=enc_t(t), rhs=enc_t(t),
                    start=(t == 0), stop=(t == RT - 1),
                )
            for t in range(RT):
                nc.tensor.matmul(
                    psum_L[:, :], lhsT=lat_t(t), rhs=lat_t(t),
                    start=(t == 0), stop=(t == RT - 1),
                )
            nc.tensor.matmul(
                psum_G[:, :], lhsT=rsra, rhs=rsra, start=True, stop=True
            ).then_inc(s_pe, 1)

    return nc


def kernel(x, encoded, latent, decoded, rsrA):
    global _NC, LAST_RESULT
    if _NC is None:
        _NC = _build_nc()

    x = np.ascontiguousarray(x, dtype=np.float32)
    decoded = np.ascontiguousarray(decoded, dtype=np.float32)
    encoded = np.ascontiguousarray(encoded, dtype=np.float32).astype(BF)
    latent = np.ascontiguousarray(latent, dtype=np.float32).astype(BF)
    rsrA_b = np.ascontiguousarray(rsrA, dtype=np.float32).astype(BF)
    ident = np.eye(P, dtype=np.float32).astype(BF)

    def stream_pack(a):
        # [1024, 1024] -> [128, 8192]: partition p holds row p of each of the
        # 8 row-tiles, concatenated (pure layout, any permutation works for
        # the elementwise sum-of-squares)
        return np.ascontiguousarray(
            a.reshape(RT, P, D).transpose(1, 0, 2).reshape(P, W)
        )

    in_maps = []
    for c in range(N_CORES):
        sl = slice(c * R, (c + 1) * R)
        pk = np.concatenate(
            [
                encoded[sl].reshape(P, ENC_W),
                latent[sl].reshape(P, LAT_W),
                rsrA_b,
                ident,
            ],
            axis=1,
        )
        xs = stream_pack(x[sl])
        ds = stream_pack(decoded[sl])
        s8 = np.concatenate(
            [
                np.concatenate([xs[:, lo:hi], ds[:, lo:hi]], axis=1)
                for lo, hi in FCHUNKS
            ],
            axis=1,
        ).astype(F8)
        s16 = np.concatenate(
            [
                np.concatenate(
                    [xs[:, FCOLS + lo : FCOLS + hi], ds[:, FCOLS + lo : FCOLS + hi]],
                    axis=1,
                )
                for lo, hi in BCHUNKS
            ],
            axis=1,
        ).astype(BF)
        in_maps.append(
            {
                "s8": np.ascontiguousarray(s8),
                "s16": np.ascontiguousarray(s16),
                "pack": np.ascontiguousarray(pk),
            }
        )

    res = run_bass_kernel_spmd(_NC, in_maps, core_ids=list(range(N_CORES)), trace=TRACE)
    LAST_RESULT = res

    o = np.stack([r["out"] for r in res.results]).astype(np.float64)  # [8,128,12]
    recon = o[:, :, 0:7].sum()
    enc2 = o[:, :, 7].sum()
    cross = o[:, :, 8].sum()
    zsq = o[:, :, 9].sum()
    g2 = o[0, :, 10].sum()
    ra2 = o[0, :, 11].sum()

    pca_sq = enc2 - 2.0 * cross + zsq
    proj_sq = g2 - 2.0 * ra2 + float(I)
    loss = recon / B + 1.1 * pca_sq / B + 0.1 * proj_sq / (I * I)
    return np.asarray(loss, dtype=np.float32)
